# revision 1
# baseline (speedup 1.0000x reference)
"""Trainium2 Bass kernel for nn_Encoder (Tacotron2-style encoder):
3x(Conv1d K=5 + BatchNorm(eval) + ReLU) -> bidirectional LSTM (H=256/dir)
with zoneout(p=0.1, eval).

Sharding: 8 cores = 2 directions x 4 batch-groups (8 samples each).
The backward direction runs the SAME program on time-reversed input with
tap-flipped conv weights; the host reverses its output back.

Per-core pipeline:
  conv stack (fp16 matmuls, folded BN via ACT Relu epilogue, two T-half
  blocks) -> x-projections for all timesteps (fp16 matmul, fp32 accum,
  fp16 staged to HBM in gate-transposed layout) -> sequential LSTM
  recurrence in transposed layout (gates [128p, 8m, 8b]) with h kept
  fp16 as the matmul moving operand and fused scalar_tensor_tensor ops
  for the zoneout algebra.  The second T-half of the conv/xproj work is
  emitted interleaved with the first 500 recurrence steps so the PE
  fills the recurrence's idle cycles.
"""
import os
import numpy as np

import concourse.bacc as bacc
import concourse.tile as tile
import concourse.mybir as mybir
from concourse.bass_utils import run_bass_kernel_spmd
from concourse.masks import make_identity

F32 = mybir.dt.float32
F32R = mybir.dt.float32r
F16 = mybir.dt.float16
AF = mybir.ActivationFunctionType
OP = mybir.AluOpType

B, C_IN, T = 32, 80, 1000
C, H, K = 512, 256, 5
BL = 8                       # samples per core
TP = T + 4                   # padded time
P_ZO = 0.1                   # zoneout keep prob
Q_ZO = 1.0 - P_ZO
BN_EPS = 1e-5
RB = 25                      # steps per ring/out group
NJJ = 8                      # xproj 125-step blocks
DEBUG = bool(int(os.environ.get("ENC_KERNEL_DEBUG", "0")))
SKIP_CONV = bool(int(os.environ.get("ENC_SKIP_CONV", "0")))
SKIP_REC = bool(int(os.environ.get("ENC_SKIP_REC", "0")))
FAKE_PAR = bool(int(os.environ.get("ENC_FAKE_PAR", "0")))
DUAL2 = bool(int(os.environ.get("ENC_DUAL2", "1")))

_CACHE = {}


def _build():
    nc = bacc.Bacc("TRN2", target_bir_lowering=False, debug=False,
                   num_devices=8)

    x_d = nc.dram_tensor("x", [C_IN, BL, TP], F16, kind="ExternalInput")
    w0_d = nc.dram_tensor("w0", [C_IN, K, C], F16, kind="ExternalInput")
    w1_d = nc.dram_tensor("w1", [128, 4, K, C], F16, kind="ExternalInput")
    w2_d = nc.dram_tensor("w2", [128, 4, K, C], F16, kind="ExternalInput")
    bn_d = nc.dram_tensor("bn", [128, 3, 2, 4], F32, kind="ExternalInput")
    wih_d = nc.dram_tensor("wih", [128, 4, 4 * H], F16, kind="ExternalInput")
    bg_d = nc.dram_tensor("bg", [1, 4 * H], F32, kind="ExternalInput")
    whh_d = nc.dram_tensor("whh", [128, 2, 4 * H], F16, kind="ExternalInput")
    out_d = nc.dram_tensor("out", [T // RB, 128, RB * 2 * BL], F16,
                           kind="ExternalOutput")

    with tile.TileContext(nc) as tc:
        with (
            tc.tile_pool(name="const", bufs=1) as cpool,
            tc.tile_pool(name="blk", bufs=2) as blk,
            tc.tile_pool(name="cps", bufs=2, space="PSUM") as cps,
            tc.tile_pool(name="xps", bufs=2, space="PSUM") as xps,
            tc.tile_pool(name="xsb", bufs=1) as xsb,
            tc.tile_pool(name="gps", bufs=2, space="PSUM") as gps,
            tc.tile_pool(name="step", bufs=3) as sp,
            tc.tile_pool(name="ring", bufs=3) as rp,
            tc.tile_pool(name="dram", bufs=1, space="DRAM") as dp,
        ):
            # per-125-step xproj staging buffers in HBM, layout [t,m,p,b]
            xpt = [dp.tile([125, 8, 128, BL], F16, name=f"xp{j}")
                   for j in range(NJJ)]

            # ---- constants / weights in SBUF ----
            x_sb = cpool.tile([C_IN, BL, TP], F16)
            nc.sync.dma_start(x_sb[:], x_d[:])
            w0 = cpool.tile([C_IN, K, C], F16)
            nc.sync.dma_start(w0[:], w0_d[:])
            w1 = cpool.tile([128, 4, K, C], F16, tag="bigw0")
            nc.sync.dma_start(w1[:], w1_d[:])
            w2 = cpool.tile([128, 4, K, C], F16, tag="bigw1")
            nc.sync.dma_start(w2[:], w2_d[:])
            bn = cpool.tile([128, 3, 2, 4], F32)
            nc.sync.dma_start(bn[:], bn_d[:])
            wih = cpool.tile([128, 4, 4 * H], F16)
            nc.sync.dma_start(wih[:], wih_d[:])
            whh = cpool.tile([128, 2, 4 * H], F16)
            nc.sync.dma_start(whh[:], whh_d[:])
            bgate_f = sp.tile([1, 4 * H], F32, name="bgf", tag="bgf")
            nc.sync.dma_start(bgate_f[:], bg_d[:])
            bgate = cpool.tile([1, 4 * H], F32R)
            nc.vector.tensor_copy(bgate[:], bgate_f[:])
            ones_f = sp.tile([1, 128], F32, name="onesf", tag="onesf")
            nc.gpsimd.memset(ones_f[:], 1.0)
            ones = cpool.tile([1, 128], F32R)
            nc.vector.tensor_copy(ones[:], ones_f[:])
            hzero = cpool.tile([128, 2, BL], F16)
            nc.gpsimd.memset(hzero[:], 0.0)
            czero = cpool.tile([128, 2, BL], F32)
            nc.gpsimd.memset(czero[:], 0.0)
            ident = cpool.tile([128, 128], F16)
            make_identity(nc, ident[:])
            identm = cpool.tile([128, 128], F16)
            nc.vector.tensor_scalar_mul(identm[:], ident[:], -Q_ZO)

            # ---- conv stack helpers (two T-half blocks) ----
            # block tile col c <-> t = base_j + c;  base = [-6, 494]
            # layer l computes t in [start_l, start_l + 506 - 2l),
            # start_l = 0 (jh0) / 494 + 2l (jh1); edge cols zeroed.
            def conv_block_make(jh):
                ot = blk.tile([128, 4, BL, 512], F16, name=f"blk{jh}",
                              tag="blk")
                if jh == 0:
                    nc.gpsimd.memset(ot[:, :, :, 0:6], 0.0)
                else:
                    nc.gpsimd.memset(ot[:, :, :, 506:512], 0.0)
                return ot

            def conv_group(jh, l, m, b, prev, ot, base=None, t_lo=None,
                           n=None):
                if base is None:
                    base = -6 if jh == 0 else 494
                    t_lo = 0 if jh == 0 else 494 + 2 * l
                    n = 506 - 2 * l
                c_lo = t_lo - base
                nm = 4 if l > 0 else 1
                w_l = (w0, w1, w2)[l]
                ps = cps.tile([128, 506], F32, name="cps", tag="cps")
                first = True
                for q in range(nm):
                    for k in range(K):
                        if l == 0:
                            lhsT = w_l[:, k, 128 * m:128 * (m + 1)]
                            rhs = x_sb[:, b, t_lo + k:t_lo + k + n]
                        else:
                            lhsT = w_l[:, q, k, 128 * m:128 * (m + 1)]
                            rhs = prev[:, q, b, c_lo - 2 + k:c_lo - 2 + k + n]
                        nc.tensor.matmul(ps[:, 0:n], lhsT, rhs, start=first,
                                         stop=(q == nm - 1 and k == K - 1))
                        first = False
                nc.scalar.activation(ot[:, m, b, c_lo:c_lo + n], ps[:, 0:n],
                                     AF.Relu, bias=bn[:, l, 1, m:m + 1],
                                     scale=bn[:, l, 0, m:m + 1])

            def xproj_block(jh, jj, feat):
                base = -6 if jh == 0 else 494
                t0 = 500 * jh + 125 * jj
                c0 = t0 - base
                stg = xsb.tile([125, 8, 128, BL], F16, name="stg", tag="stg")
                for b in range(BL):
                    for nn in range(2):
                        ps = xps.tile([125, 512], F32, name="xps", tag="xps")
                        for q in range(4):
                            nc.tensor.matmul(
                                ps[:],
                                feat[:, q, b, c0:c0 + 125],
                                wih[:, q, 512 * nn:512 * (nn + 1)],
                                start=(q == 0), stop=False)
                        nc.tensor.matmul(
                            ps[:],
                            ones[:, 0:125],
                            bgate[:, 512 * nn:512 * (nn + 1)],
                            start=False, stop=True)
                        nc.scalar.activation(
                            stg[:, 4 * nn:4 * (nn + 1), :, b],
                            ps[:].rearrange("t (m p) -> t m p", p=128),
                            AF.Copy)
                nc.sync.dma_start(xpt[4 * jh + jj][:], stg[:])

            # ---- prologue: quarter conv block covering t<134 so the
            # recurrence can start almost immediately; both conv halves
            # are then re-emitted in full, paced into the recurrence's
            # idle PE cycles (overlap regions recompute identical values).
            segA, segB = [], []
            if not SKIP_CONV:
                prevq = None
                for l in range(3):
                    otq = blk.tile([128, 4, BL, 144], F16, name=f"q{l}",
                                   tag="blkq", bufs=2)
                    nc.gpsimd.memset(otq[:, :, :, 0:6], 0.0)
                    for m in range(4):
                        for b in range(BL):
                            conv_group(0, l, m, b, prevq, otq,
                                       base=-6, t_lo=0, n=138 - 2 * l)
                    prevq = otq
                xproj_block(0, 0, prevq)

                tiles1 = {}

                def mk_block(jh, l):
                    def f():
                        tiles1[(jh, l)] = conv_block_make(jh)
                    return f

                def mk_group(jh, l, m, b):
                    def f():
                        conv_group(jh, l, m, b, tiles1.get((jh, l - 1)),
                                   tiles1[(jh, l)])
                    return f

                def mk_xproj(jh, jj):
                    def f():
                        xproj_block(jh, jj, tiles1[(jh, 2)])
                    return f

                for jh, seg in ((0, segA), (1, segB)):
                    for l in range(3):
                        seg.append(mk_block(jh, l))
                        for m in range(4):
                            for b in range(BL):
                                seg.append(mk_group(jh, l, m, b))
                    for jj in range(4):
                        if jh == 1:
                            seg.append(mk_xproj(jh, jj))
                segA.append(mk_xproj(0, 1))
                late = {9: mk_xproj(0, 2), 14: mk_xproj(0, 3)}

            # ---- recurrence ----
            # t < T/2 : single chain (B=8), conv second-half interleaved
            # t >= T/2: two half-batch chains (B=4), software-pipelined
            #          with a half-step skew to hide the serial latency.
            n_grp = (T if not SKIP_REC else 0) // RB
            n_ov = n_grp // 2

            xr_tiles = {}

            def get_xr(g):
                if g not in xr_tiles:
                    xr = rp.tile([128, RB, 8, BL], F16, name="xr", tag="xr", bufs=2)
                    blkj = (g * RB) // 125
                    toff = g * RB - 125 * blkj
                    nc.sync.dma_start(
                        xr[:],
                        xpt[blkj][toff:toff + RB]
                        .rearrange("t m p b -> p t m b"))
                    xr_tiles[g] = xr
                return xr_tiles[g]

            def emit_mm(t, bs, n_b, h_ap, tag):
                g, s = t // RB, t % RB
                xr = get_xr(g)
                halves = []
                for hf in range(2):
                    pg = gps.tile([128, 4, n_b], F32, name=f"pg{hf}",
                                  tag=tag, bufs=4)
                    nc.tensor.matmul(pg[:], ident[:],
                                     xr[:, s, 4 * hf:4 * hf + 4,
                                        bs:bs + n_b],
                                     start=True, stop=False)
                    for mm in range(4):
                        m = 4 * hf + mm
                        for kc in range(2):
                            nc.tensor.matmul(
                                pg[:, mm, :],
                                whh[:, kc, 128 * m:128 * (m + 1)],
                                h_ap[:, kc, :],
                                start=False, stop=(mm == 3 and kc == 1))
                    halves.append(pg)
                return halves

            def emit_elem(pg, h_ap, c_ap, hr_out, n_b, sfx):
                pg_lo, pg_hi = pg
                # sigmoid over all gates; g-gate cols pre-doubled host-side
                # so tanh(g) = 2*sio_g - 1.  lo half = (i, g), hi = (f, o).
                slo = sp.tile([128, 4, n_b], F16, name="slo",
                              tag="slo" + sfx)
                nc.scalar.activation(slo[:], pg_lo[:], AF.Sigmoid)
                shi = sp.tile([128, 4, n_b], F16, name="shi",
                              tag="shi" + sfx)
                nc.scalar.activation(shi[:], pg_hi[:], AF.Sigmoid)
                v1 = sp.tile([128, 2, n_b], F16, name="v1", tag="v1" + sfx)
                nc.vector.scalar_tensor_tensor(
                    v1[:], slo[:, 2:4, :], 2.0 * Q_ZO, slo[:, 0:2, :],
                    OP.mult, OP.mult)
                v2 = sp.tile([128, 2, n_b], F16, name="v2", tag="v2" + sfx)
                nc.vector.scalar_tensor_tensor(
                    v2[:], shi[:, 0:2, :], Q_ZO, c_ap, OP.mult, OP.mult)
                t1 = sp.tile([128, 2, n_b], F16, name="t1", tag="t1" + sfx)
                nc.vector.tensor_add(t1[:], v1[:], v2[:])
                w_t = sp.tile([128, 2, n_b], F32, name="w", tag="w" + sfx)
                nc.vector.scalar_tensor_tensor(
                    w_t[:], slo[:, 0:2, :], -Q_ZO, t1[:], OP.mult, OP.add)
                c_new = sp.tile([128, 2, n_b], F32, name="c", tag="c" + sfx)
                nc.vector.scalar_tensor_tensor(
                    c_new[:], c_ap, P_ZO, w_t[:], OP.mult, OP.add)
                tc2 = sp.tile([128, 2, n_b], F16, name="tc2",
                              tag="tc2" + sfx)
                nc.scalar.activation(tc2[:], w_t[:], AF.Tanh,
                                     scale=1.0 / Q_ZO)
                u = sp.tile([128, 2, n_b], F16, name="u", tag="u" + sfx)
                nc.vector.scalar_tensor_tensor(
                    u[:], shi[:, 2:4, :], Q_ZO, tc2[:], OP.mult, OP.mult)
                nc.vector.scalar_tensor_tensor(
                    hr_out, h_ap, P_ZO, u[:], OP.mult, OP.add)
                return c_new[:]

            # --- phase 1: single chain, conv work interleaved ---
            # segA (first conv half + jj1-3) due by group 5; segB (second
            # half + jj4-7) due by group n_ov.
            nga = int(os.environ.get("ENC_NGA", "5"))
            c_prev = czero[:]
            hring = None
            def emit_work(g):
                if not SKIP_CONV and g in late:
                    late[g]()
                if segA and g < nga:
                    for item in segA[(g * len(segA)) // nga:
                                     ((g + 1) * len(segA)) // nga]:
                        item()
                if segB and nga <= g < n_ov:
                    gb, nb = g - nga, n_ov - nga
                    for item in segB[(gb * len(segB)) // nb:
                                     ((gb + 1) * len(segB)) // nb]:
                        item()

            n_p1 = n_ov if DUAL2 else n_grp
            for g in range(min(n_p1, n_grp)):
                hring_prev = hring
                hring = rp.tile([128, RB, 2, BL], F16, name="hr",
                                tag="hring")
                for s in range(RB):
                    t = g * RB + s
                    if t == 0:
                        h_ap = hzero[:]
                    elif s == 0:
                        h_ap = hring_prev[:, RB - 1, :, :]
                    else:
                        h_ap = hring[:, s - 1, :, :]
                    if FAKE_PAR:
                        h_ap = hzero[:]
                    pg = emit_mm(t, 0, BL, h_ap, "gps")
                    c_prev = emit_elem(pg, h_ap, c_prev,
                                       hring[:, s, :, :], BL, "")
                nc.sync.dma_start(
                    out_d[g],
                    hring[:].rearrange("p t kc b -> p (t kc b)"))
                emit_work(g)

            # --- phase 2: dual half-batch chains, half-step skew ---
            if DUAL2 and n_grp > n_ov:
                t2 = n_ov * RB
                BH = BL // 2
                ch_c = [c_prev[:, :, 0:BH], c_prev[:, :, BH:BL]]
                ch_h = [hring[:, RB - 1, :, 0:BH],
                        hring[:, RB - 1, :, BH:BL]]
                ch_hr = [None, None]
                ch_pg = [None, None]

                def hr_tile(g, ch):
                    if ch_hr[ch] is None or ch_hr[ch][0] != g:
                        tile_ = rp.tile([128, RB, 2, BH], F16,
                                        name=f"hrd{ch}", tag=f"hrd{ch}")
                        ch_hr[ch] = (g, tile_)
                    return ch_hr[ch][1]

                def flush_out(g, ch):
                    tile_ = ch_hr[ch][1]
                    ov = out_d[g].rearrange("p (t kc b) -> p t kc b",
                                            kc=2, b=BL)
                    for kc in range(2):
                        nc.sync.dma_start(
                            ov[:, :, kc, ch * BH:(ch + 1) * BH],
                            tile_[:, :, kc, :])

                def mm_step(ch, t):
                    g, s = t // RB, t % RB
                    h_in = hzero[:, :, 0:BH] if FAKE_PAR else ch_h[ch]
                    ch_pg[ch] = emit_mm(t, ch * BH, BH, h_in, "gps")

                def elem_step(ch, t):
                    g, s = t // RB, t % RB
                    hr = hr_tile(g, ch)
                    out_slot = hr[:, s, :, :]
                    ch_c[ch] = emit_elem(ch_pg[ch], ch_h[ch], ch_c[ch],
                                         out_slot, BH, f"d{ch}")
                    ch_h[ch] = out_slot
                    if s == RB - 1:
                        flush_out(g, ch)

                mm_step(0, t2)
                for t in range(t2, T):
                    mm_step(1, t)
                    elem_step(0, t)
                    if t + 1 < T:
                        mm_step(0, t + 1)
                    elem_step(1, t)

    nc.compile()
    return nc


def _prep_core(inputs, core):
    f32 = np.float32
    fwd = core < 4
    tag = "f" if fwd else "b"
    bsl = slice(8 * (core % 4), 8 * (core % 4) + 8)
    # gate order [i, g, f, o] so the (i, g) half of the gates can be
    # consumed as soon as the first half of the recurrent matmuls lands
    perm = np.concatenate([np.arange(0, H), np.arange(2 * H, 3 * H),
                           np.arange(H, 2 * H), np.arange(3 * H, 4 * H)])

    x = np.asarray(inputs["x"], f32)[bsl].transpose(1, 0, 2)   # [Cin, 8, T]
    if not fwd:
        x = x[:, :, ::-1]
    xp = np.zeros((C_IN, BL, TP), f32)
    xp[:, :, 2:2 + T] = x

    d = {"x": xp.astype(np.float16)}

    bn = np.zeros((128, 3, 2, 4), f32)
    for l in range(3):
        cw = np.asarray(inputs[f"cw{l}"], f32)
        if not fwd:
            cw = cw[:, :, ::-1]
        s = np.asarray(inputs[f"bg{l}"], f32) / np.sqrt(
            np.asarray(inputs[f"bv{l}"], f32) + BN_EPS)
        bias = ((np.asarray(inputs[f"cb{l}"], f32)
                 - np.asarray(inputs[f"bm{l}"], f32)) * s
                + np.asarray(inputs[f"bb{l}"], f32))
        bn[:, l, 0, :] = s.reshape(4, 128).T
        bn[:, l, 1, :] = bias.reshape(4, 128).T
        wt = cw.transpose(1, 2, 0)                 # [cin, K, C]
        if l == 0:
            d["w0"] = np.ascontiguousarray(wt).astype(np.float16)
        else:
            d[f"w{l}"] = np.ascontiguousarray(
                wt.reshape(4, 128, K, C).transpose(1, 0, 2, 3)
            ).astype(np.float16)
    d["bn"] = bn

    wih = np.asarray(inputs[f"wih_{tag}"], f32)[perm]          # [1024, 512]
    whh = np.asarray(inputs[f"whh_{tag}"], f32)[perm]          # [1024, 256]
    bg = (np.asarray(inputs[f"bih_{tag}"], f32)
          + np.asarray(inputs[f"bhh_{tag}"], f32))[perm]
    # g-gate rows doubled: kernel computes tanh(g) as 2*sigmoid(2g)-1
    wih = wih.copy(); whh = whh.copy(); bg = bg.copy()
    wih[H:2 * H] *= 2.0
    whh[H:2 * H] *= 2.0
    bg[H:2 * H] *= 2.0
    d["wih"] = np.ascontiguousarray(
        wih.T.reshape(4, 128, 4 * H).transpose(1, 0, 2)).astype(np.float16)
    d["whh"] = np.ascontiguousarray(
        whh.T.reshape(2, 128, 4 * H).transpose(1, 0, 2)).astype(np.float16)
    d["bg"] = bg.reshape(1, 4 * H)
    return d


def kernel(**inputs):
    if "nc" not in _CACHE:
        _CACHE["nc"] = _build()
    nc = _CACHE["nc"]
    in_maps = [_prep_core(inputs, c) for c in range(8)]
    res = run_bass_kernel_spmd(nc, in_maps, list(range(8)))
    _CACHE["last"] = res
    out = np.empty((B, T, 2 * H), np.float32)
    for c in range(8):
        bsl = slice(8 * (c % 4), 8 * (c % 4) + 8)
        arr = np.asarray(res.results[c]["out"], np.float32)
        arr = arr.reshape(T // RB, 128, RB, 2, BL)
        h = arr.transpose(4, 0, 2, 3, 1).reshape(BL, T, H)
        if c < 4:
            out[bsl, :, :H] = h
        else:
            out[bsl, :, H:] = h[:, ::-1, :]
    return out

